# revision 12
# baseline (speedup 1.0000x reference)
"""MeshUpdateNet (EdgeConv message passing + MLP decoder) on 8 Trainium2
NeuronCores via Bass/Tile.

Sharding: nodes dealt round-robin by descending degree to the 8 cores
(no collectives); each core owns NC nodes and all edges pointing at them.

v2 layout (strip-major): node columns are cut into 1024-wide strips; for
each strip all covering ranks are processed consecutively. Rank widths
are padded to 512 so every edge tile is 512 or 1024 wide. Per tile:
  mm1: ps1 = w1m^T s (K=128 zero-padded bf16: uniform matmul shape -
     alternating contraction shapes, incl. fp8 DoubleRow for mm1, pins
     the PE at its slow p-state, measured 615ns vs 459ns per 512-col)
  relu drain: ACT activation (lane 'A') or DVE tensor_scalar ('V'),
     split by a static pattern to balance the two psum-drain engines
  mm2: ps2 = w2^T h1 (bf16)
  max drain, one of (static pattern):
     'S': aggb = max(ps2, aggb) directly on DVE
     'C': ACT copies ps2 -> t (bf16), DVE merges aggb = max(t, aggb) in
          2x_1p mode (all-bf16 sbuf) - half the DVE cost of a direct
          drain, so a sprinkle of 'C' tiles rebalances ACT vs DVE
          (gpsimd/Pool supports no tensor_tensor max, DMA cannot read
          psum, and no instruction may read two psum operands - ACT and
          DVE are the only psum drains available)
The psum-drain work (2 fp32 columns per edge, locked to 1 col/cycle on
ACT/DVE - psum has no 2x modes) is the roofline; the three-lane split
plus a software-pipelined issue order (mm1 of tile i ahead of mm2 of
tile i-1, PE never starves) targets it.

agg is kept in bf16 (plenty of precision: the decoder output is a small
perturbation of pos). Nodes with no edges are patched on the host with
the closed-form output. Tail (enc/dec MLP) runs per 2048-node group with
psum-packed w5 matmuls, reading agg via 4x-mode tensor_scalar.
"""
import sys

sys.path.insert(0, '/opt/trn_rl_repo')

import numpy as np
import ml_dtypes

import concourse.bass as bass
import concourse.tile as tile
from concourse import bacc, mybir
from concourse import bass_utils

F32 = mybir.dt.float32
BF16 = mybir.dt.bfloat16
FP8 = mybir.dt.float8e4
BF = ml_dtypes.bfloat16
F8 = ml_dtypes.float8_e4m3

N_CORES = 8
STRIP_W = 1024     # node-column strip width = edge tile width
RANK_PAD = 512     # rank widths padded to this multiple
MM_W = 512         # max matmul moving free dim (one psum bank)
CHUNK = 8192       # stream DMA chunk (cols)
NODE_W = 512       # tail node-tile width
GROUP = 4          # node tiles packed per psum group in the tail
WARMUP_MM = 10     # gapless matmul chain to ramp the PE p-state

RELU_V_PERIOD = 0   # 0: all relus on ACT; k>0: every k-th relu on DVE
DRAIN_C_PERIOD = 16  # every k-th max-drain via ACT-copy + DVE bf16 merge


def make_schedule(deg, n_nodes):
    """Strip-major (all-cores-common) edge tiling from global degrees."""
    nodes_sorted = np.argsort(-deg, kind='stable')
    deg_sorted = deg[nodes_sorted]
    d_max = int(deg_sorted[0]) if len(deg_sorted) else 0
    # M_r = #nodes with degree > r ; N_r = common per-core rank width
    M = np.searchsorted(-deg_sorted, -(np.arange(d_max) + 1), side='right')
    NC = n_nodes // N_CORES
    N_r = -(-M // N_CORES)
    N_rp = -(-N_r // RANK_PAD) * RANK_PAD          # padded rank widths
    n_strips = -(-NC // (GROUP * NODE_W)) * (GROUP * NODE_W) // STRIP_W
    NC_pad = n_strips * STRIP_W
    np.minimum(N_rp, NC_pad, out=N_rp)

    # tiles: (strip, rank, slot_offset, col0, width, drain_mode, relu_lane)
    # Strips are processed in PAIRS (2g, 2g+1) with their ranks interleaved:
    # consecutive drains then target different agg columns, so the DVE's
    # per-strip read-modify-write chain never stalls on its own writeback.
    tiles = []
    seq = 0
    for g2 in range(0, n_strips, 2):
        pair_tiles = []
        for s in (g2, g2 + 1):
            if s >= n_strips:
                continue
            c0 = s * STRIP_W
            covering = np.nonzero(N_rp > c0)[0]
            st = []
            for r in covering:
                W = int(min(STRIP_W, N_rp[r] - c0))
                st.append([s, int(r), 0, c0, W, 'S'])
            pair_tiles.append(st)
        merged = []
        k = max((len(st) for st in pair_tiles), default=0)
        for i in range(k):
            for st in pair_tiles:
                if i < len(st):
                    merged.append(st[i])
        for t in merged:
            t[2] = seq
            seq += t[4]
        tiles.extend(merged)
        seq = -(-seq // STRIP_W) * STRIP_W         # pair base 1024-aligned
    L = -(-seq // CHUNK) * CHUNK
    for i, t in enumerate(tiles):
        if DRAIN_C_PERIOD and i % DRAIN_C_PERIOD == DRAIN_C_PERIOD - 1:
            t[5] = 'C'
        t.append('V' if RELU_V_PERIOD and
                 i % RELU_V_PERIOD == RELU_V_PERIOD - 1 else 'A')
    n_ntiles = NC_pad // NODE_W
    n_groups = n_ntiles // GROUP
    return dict(nodes_sorted=nodes_sorted, deg_sorted=deg_sorted, d_max=d_max,
                NC=NC, NC_pad=NC_pad, N_r=N_r, N_rp=N_rp, L=L, tiles=tiles,
                n_strips=n_strips, n_ntiles=n_ntiles, n_groups=n_groups)


def build_nc(sched):
    NC_pad, L = sched['NC_pad'], sched['L']
    tiles = sched['tiles']
    n_groups = sched['n_groups']
    GPC = n_groups * NODE_W

    nc = bacc.Bacc("TRN2", target_bir_lowering=False, debug=False,
                   enable_asserts=False, num_devices=N_CORES)

    xs_d = nc.dram_tensor("xs", [6, L], BF16, kind="ExternalInput").ap()
    pospk_d = nc.dram_tensor("pospk", [99, GPC], F32, kind="ExternalInput").ap()
    w1m_d = nc.dram_tensor("w1m", [128, 128], BF16, kind="ExternalInput").ap()
    w2_d = nc.dram_tensor("w2", [128, 128], BF16, kind="ExternalInput").ap()
    w3_d = nc.dram_tensor("w3", [128, 128], BF16, kind="ExternalInput").ap()
    w4_d = nc.dram_tensor("w4", [128, 128], BF16, kind="ExternalInput").ap()
    w5_d = nc.dram_tensor("w5", [128, 3], BF16, kind="ExternalInput").ap()
    b1_d = nc.dram_tensor("b1", [128, 1], F32, kind="ExternalInput").ap()
    b2_d = nc.dram_tensor("b2", [128, 1], F32, kind="ExternalInput").ap()
    b4p_d = nc.dram_tensor("b4p", [128, 1], F32, kind="ExternalInput").ap()
    b5pk_d = nc.dram_tensor("b5pk", [99, 1], F32, kind="ExternalInput").ap()
    out_d = nc.dram_tensor("outpk", [99, GPC], F32, kind="ExternalOutput").ap()

    RELU = mybir.ActivationFunctionType.Relu
    TANH = mybir.ActivationFunctionType.Tanh
    COPY = mybir.ActivationFunctionType.Copy
    ADD = mybir.AluOpType.add
    MAX = mybir.AluOpType.max
    MULT = mybir.AluOpType.mult

    with tile.TileContext(nc) as tc:
        with (
            tc.tile_pool(name="const", bufs=1) as cp,
            tc.tile_pool(name="aggp", bufs=1) as aggp,
            tc.tile_pool(name="stream", bufs=1) as sp,
            tc.tile_pool(name="hpool", bufs=3) as hp,
            tc.tile_pool(name="tpool", bufs=3) as tp,
            tc.tile_pool(name="gio", bufs=3) as gio,
        ):
            # constants needed early
            w2_s = cp.tile([128, 128], BF16)
            nc.sync.dma_start(w2_s[:], w2_d[:])
            w1m_s = cp.tile([128, 128], BF16)
            nc.sync.dma_start(w1m_s[:], w1m_d[:])
            b1_s = cp.tile([128, 1], F32)
            nc.sync.dma_start(b1_s[:], b1_d[:])

            # PE warm-up first: gapless matmul chain in its own psum scope;
            # the p-state ramp needs >3us of uninterrupted execution, and the
            # chain must start ASAP so it hands off to the stream hot.
            warm_rhs = cp.tile([128, 512], BF16)
            nc.scalar.memzero(warm_rhs[:])
            with tc.tile_pool(name="psW", bufs=4, space="PSUM") as pW:
                for i in range(WARMUP_MM):
                    wps = pW.tile([128, 512], F32, tag="warm")
                    nc.tensor.matmul(wps[:], w2_s[:], warm_rhs[:],
                                     start=True, stop=True)

            # Stream tiles: rows 0-5 carry the DMA'd [xi;xj] stream; rows
            # 6-127 are zeroed once and never rewritten, so mm1 contracts
            # over K=128 with a zero-padded w1m. bf16 memsets run in the
            # DVE's 4x mode (~1.3us each), so they all go there, ahead of
            # the agg init; the gpsimd queue stays free for DMA configs.
            ch_bufs = []
            for bi in range(3):
                chb = sp.tile([128, CHUNK], BF16, tag=f"xs{bi}", name=f"xs{bi}")
                nc.vector.memset(chb[:], 0.0)
                ch_bufs.append(chb)

            agg = aggp.tile([128, NC_pad], BF16)
            nc.vector.memset(agg[:], -1e30)

            with (
                tc.tile_pool(name="ps1", bufs=2, space="PSUM") as p1,
                tc.tile_pool(name="ps2", bufs=2, space="PSUM") as p2,
            ):
                # Chunk DMAs are issued just-in-time from the (otherwise
                # idle) gpsimd sequencer: the SP sequencer needs ~2.4us per
                # DMA config, so 28 upfront configs serialized ~67us of
                # startup with every engine waiting on the queue.
                n_chunks = L // CHUNK
                chunk_tiles = {}
                issued = [0]

                def issue_chunks(upto):
                    while issued[0] <= min(upto, n_chunks - 1):
                        ci = issued[0]
                        ch = ch_bufs[ci % 3]
                        nc.gpsimd.dma_start(
                            ch[:6, :], xs_d[:, ci * CHUNK:(ci + 1) * CHUNK])
                        chunk_tiles[ci] = ch
                        issued[0] += 1

                issue_chunks(2)

                T = len(tiles)
                ps1_of = {}

                def issue_mm1(i):
                    (s, r, so, c0, W, dm, rl) = tiles[i]
                    i2 = min(i + 8, T - 1)
                    issue_chunks((tiles[i2][2] + tiles[i2][4]) // CHUNK)
                    ps1_t = p1.tile([128, STRIP_W], F32, tag="p1", name="p1")
                    ci, off = so // CHUNK, so % CHUNK
                    ch = chunk_tiles[ci]
                    for h in range(0, W, MM_W):
                        nc.tensor.matmul(ps1_t[:, h:h + MM_W], w1m_s[:],
                                         ch[:, off + h: off + h + MM_W],
                                         start=True, stop=True)
                    ps1_of[i] = ps1_t

                h1_of = {}

                def issue_relu(j):
                    (s, r, so, c0, W, dm, rl) = tiles[j]
                    ps1_t = ps1_of.pop(j)
                    h1 = hp.tile([128, STRIP_W], BF16, tag="h1", name="h1")
                    if rl == 'A':
                        nc.scalar.activation(h1[:, :W], ps1_t[:, :W], RELU,
                                             bias=b1_s[:, 0:1])
                    else:
                        nc.vector.tensor_scalar(
                            out=h1[:, :W], in0=ps1_t[:, :W],
                            scalar1=b1_s[:, 0:1], scalar2=0.0,
                            op0=ADD, op1=MAX)
                    h1_of[j] = h1

                def issue_mm2_drain(j):
                    (s, r, so, c0, W, dm, rl) = tiles[j]
                    h1 = h1_of.pop(j)
                    ps2_t = p2.tile([128, STRIP_W], F32, tag="p2", name="p2")
                    for h in range(0, W, MM_W):
                        nc.tensor.matmul(ps2_t[:, h:h + MM_W], w2_s[:],
                                         h1[:, h:h + MM_W],
                                         start=True, stop=True)
                    if dm == 'S':
                        nc.vector.tensor_tensor(
                            out=agg[:, c0:c0 + W], in0=ps2_t[:, :W],
                            in1=agg[:, c0:c0 + W], op=MAX)
                    else:  # 'C': ACT copy to bf16, DVE 2x merge
                        t_t = tp.tile([128, STRIP_W], BF16, tag="t", name="t")
                        nc.scalar.activation(t_t[:, :W], ps2_t[:, :W], COPY)
                        nc.vector.tensor_tensor(
                            out=agg[:, c0:c0 + W], in0=t_t[:, :W],
                            in1=agg[:, c0:c0 + W], op=MAX)

                for i in range(T + 2):
                    if i < T:
                        issue_mm1(i)
                    if i >= 1 and i - 1 < T:
                        issue_relu(i - 1)
                    if i >= 2:
                        issue_mm2_drain(i - 2)

            # tail constants
            w3_s = cp.tile([128, 128], BF16)
            nc.sync.dma_start(w3_s[:], w3_d[:])
            w4_s = cp.tile([128, 128], BF16)
            nc.sync.dma_start(w4_s[:], w4_d[:])
            w5_s = cp.tile([128, 3], BF16)
            nc.sync.dma_start(w5_s[:], w5_d[:])
            b2_s = cp.tile([128, 1], F32)
            nc.sync.dma_start(b2_s[:], b2_d[:])
            b4p_s = cp.tile([128, 1], F32)
            nc.sync.dma_start(b4p_s[:], b4p_d[:])
            b5pk_s = cp.tile([99, 1], F32)
            nc.sync.dma_start(b5pk_s[:], b5pk_d[:])

            with (
                tc.tile_pool(name="psT", bufs=2, space="PSUM") as pT,
                tc.tile_pool(name="psG", bufs=2, space="PSUM") as pG,
                tc.tile_pool(name="wrk", bufs=4) as wp,
            ):
                for g in range(n_groups):
                    pos_g = gio.tile([99, NODE_W], F32, tag="pos", name="pos")
                    nc.gpsimd.dma_start(
                        pos_g[:], pospk_d[:, g * NODE_W:(g + 1) * NODE_W])
                    ps5 = pG.tile([99, NODE_W], F32, tag="p5", name="p5")
                    nc.vector.memset(ps5[:], 0.0)
                    for j in range(GROUP):
                        t = g * GROUP + j
                        c0 = t * NODE_W
                        # r3 = relu(agg + b2) in bf16 (4x-mode tensor_scalar)
                        r3 = wp.tile([128, NODE_W], BF16, tag="r3", name="r3")
                        nc.vector.tensor_scalar(
                            out=r3[:], in0=agg[:, c0:c0 + NODE_W],
                            scalar1=b2_s[:, 0:1], scalar2=0.0,
                            op0=ADD, op1=MAX)
                        ps3 = pT.tile([128, NODE_W], F32, tag="p3", name="p3")
                        nc.tensor.matmul(ps3[:], w3_s[:], r3[:],
                                         start=True, stop=True)
                        e4 = wp.tile([128, NODE_W], BF16, tag="e4", name="e4")
                        if j % 2 == 0:
                            nc.scalar.activation(e4[:], ps3[:], COPY)
                        else:
                            nc.vector.tensor_copy(e4[:], ps3[:])
                        ps4 = pT.tile([128, NODE_W], F32, tag="p4", name="p4")
                        nc.tensor.matmul(ps4[:], w4_s[:], e4[:],
                                         start=True, stop=True)
                        r5 = wp.tile([128, NODE_W], BF16, tag="r5", name="r5")
                        if j % 2 == 0:
                            nc.vector.tensor_scalar(
                                out=r5[:], in0=ps4[:],
                                scalar1=b4p_s[:, 0:1], scalar2=0.0,
                                op0=ADD, op1=MAX)
                        else:
                            nc.scalar.activation(r5[:], ps4[:], RELU,
                                                 bias=b4p_s[:, 0:1])
                        nc.tensor.matmul(ps5[32 * j:32 * j + 3, :], w5_s[:],
                                         r5[:], start=True, stop=True,
                                         tile_position=(0, 32 * j))
                    s_t = wp.tile([99, NODE_W], F32, tag="s", name="s")
                    nc.scalar.activation(s_t[:], ps5[:], TANH,
                                         bias=b5pk_s[:, 0:1])
                    out_g = gio.tile([99, NODE_W], F32, tag="og", name="og")
                    nc.vector.scalar_tensor_tensor(
                        out=out_g[:], in0=s_t[:], scalar=0.1, in1=pos_g[:],
                        op0=MULT, op1=ADD)
                    nc.gpsimd.dma_start(
                        out_d[:, g * NODE_W:(g + 1) * NODE_W], out_g[:])
    nc.compile()
    return nc


def make_inputs(x, pos, w1, b1, w2, b2, w3, b3, w4, b4, w5, b5,
                src, dst, sched):
    n_nodes = x.shape[0]
    E = src.shape[0]
    NC, NC_pad, L = sched['NC'], sched['NC_pad'], sched['L']
    tiles = sched['tiles']
    nodes_sorted = sched['nodes_sorted']
    n_groups = sched['n_groups']
    GPC = n_groups * NODE_W

    order = np.argsort(dst, kind='stable')
    src_sorted = src[order]
    deg = np.bincount(dst, minlength=n_nodes)
    starts = np.zeros(n_nodes + 1, np.int64)
    np.cumsum(deg, out=starts[1:])

    # msg @ w1 = [xi ; xj-xi] @ w1 = [xi ; xj] @ [[w1a-w1b]; [w1b]]
    w1a, w1b = w1[:3], w1[3:]
    w1m = np.zeros((128, 128), np.float32)
    w1m[:6] = np.vstack([w1a - w1b, w1b])
    w1m = w1m.astype(BF)
    b4p = (b3 @ w4 + b4).astype(np.float32).reshape(128, 1)   # fold b3
    b5pk = np.zeros((99, 1), np.float32)
    for j in range(GROUP):
        b5pk[32 * j:32 * j + 3, 0] = b5

    common = dict(
        w1m=w1m, w2=w2.astype(BF), w3=w3.astype(BF), w4=w4.astype(BF),
        w5=w5.astype(BF), b1=b1.reshape(128, 1).astype(np.float32),
        b2=b2.reshape(128, 1).astype(np.float32), b4p=b4p, b5pk=b5pk)

    in_maps = []
    for c in range(N_CORES):
        loc_nodes = nodes_sorted[c::N_CORES]
        loc_deg = deg[loc_nodes]
        loc_start = starts[loc_nodes]
        xi_loc = x[loc_nodes]
        xs = np.zeros((6, L), BF)
        for (s, r, so, c0, W, dm, rl) in tiles:
            cols = np.minimum(np.arange(c0, c0 + W), NC - 1)
            has = loc_deg[cols] > r
            # pad slots duplicate the node's first edge (max-idempotent);
            # deg-0 nodes gather garbage and are patched on the host
            idx = np.where(has, loc_start[cols] + r, loc_start[cols])
            np.minimum(idx, E - 1, out=idx)
            xs[0:3, so:so + W] = xi_loc[cols].T.astype(BF)
            xs[3:6, so:so + W] = x[src_sorted[idx]].T.astype(BF)
        # pack pos tiles 4-per-group into partition strips 32j..32j+2
        pos_t = np.zeros((3, NC_pad), np.float32)
        pos_t[:, :NC] = pos[loc_nodes].T
        ptiles = pos_t.reshape(3, n_groups * GROUP, NODE_W)
        pospk = np.zeros((99, n_groups, NODE_W), np.float32)
        for j in range(GROUP):
            pospk[32 * j:32 * j + 3] = ptiles[:, j::GROUP, :]
        in_maps.append(dict(xs=xs, pospk=pospk.reshape(99, GPC), **common))
    return in_maps


def unpack_outputs(results, sched, pos, deg, w3, b3, w4, b4, w5, b5):
    NC, NC_pad = sched['NC'], sched['NC_pad']
    nodes_sorted = sched['nodes_sorted']
    n_groups = sched['n_groups']
    n = len(nodes_sorted)
    out_full = np.zeros((n, 3), np.float32)
    for c in range(N_CORES):
        outpk = results[c]['outpk'].reshape(99, n_groups, NODE_W)
        tiles_o = np.zeros((3, n_groups * GROUP, NODE_W), np.float32)
        for j in range(GROUP):
            tiles_o[:, j::GROUP, :] = outpk[32 * j:32 * j + 3]
        out_t = tiles_o.reshape(3, -1)[:, :NC]
        out_full[nodes_sorted[c::N_CORES]] = out_t.T
    deg0 = deg == 0
    if deg0.any():
        # closed form for isolated nodes: agg = 0 -> enc = b3
        enc0 = b3
        dec0 = np.maximum(enc0 @ w4 + b4, 0.0) @ w5 + b5
        out_full[deg0] = pos[deg0] + 0.1 * np.tanh(dec0)
    return out_full


def run(inputs, trace=False, tmpdir=None):
    x = np.asarray(inputs['x'], np.float32)
    pos = np.asarray(inputs['pos'], np.float32)
    ei = np.asarray(inputs['edge_index'])
    src = ei[0].astype(np.int64)
    dst = ei[1].astype(np.int64)
    deg = np.bincount(dst, minlength=x.shape[0])
    sched = make_schedule(deg, x.shape[0])
    nc = build_nc(sched)
    args = [np.asarray(inputs[k], np.float32) for k in
            ('w1', 'b1', 'w2', 'b2', 'w3', 'b3', 'w4', 'b4', 'w5', 'b5')]
    in_maps = make_inputs(x, pos, *args, src, dst, sched)
    res = bass_utils.run_bass_kernel_spmd(
        nc, in_maps, core_ids=list(range(N_CORES)), trace=trace, tmpdir=tmpdir)
    w3_, b3_, w4_, b4_, w5_, b5_ = args[4:]
    out = unpack_outputs(res.results, sched, pos, deg,
                         w3_, b3_, w4_, b4_, w5_, b5_)
    return out, res


def kernel(**inputs):
    out, _ = run(inputs, trace=False)
    return out
